# revision 14
# baseline (speedup 1.0000x reference)
"""Raw-Bacc v7: single-ring 4KB-line loads, DVE pre-add (+v) before PE
transpose, 5-chunk pipeline (512,512,512,256,256) with per-chunk
output DMAs alternating across both HWDGE rings.

out[n, c] = pf[c, n] + v[c],  v = Wv @ age + bv

wvx host-packed [128, 129] f32: cols 0:64 = Wv, 64:128 = age bcast,
col 128 = bv.
v chain (DVE): vcol = reduce_sum(Wv*age_bc) + bv   [128, 1]
Per chunk k: DVE tensor_scalar(pft_k += vcol)  (v is per-partition in
[c, n] layout), PE transposes 128-col blocks into psum bank k, DVE
copies bank -> osb_k, out DMA issued immediately.
Loads: pf as two 512KB half DMAs (4KB/partition lines) on the Sync
ring only; wvx+iden tiny on the Scalar ring. Outs: sync gets chunks
0,2; scalar gets 1,3,4.
"""

import numpy as np

N_CORES = 8
B, C, D, H, W = 1, 128, 16, 32, 32
N = D * H * W
NSH = N // N_CORES       # 2048
AGE = 64
HLF = NSH // 2           # 1024 cols per load DMA
CHUNKS = [512, 512, 512, 256, 256]
NCK = len(CHUNKS)
COFF = [0, 512, 1024, 1536, 1792]


def build_nc():
    import concourse.bacc as bacc
    import concourse.mybir as mybir
    from contextlib import ExitStack

    f32 = mybir.dt.float32
    nc = bacc.Bacc(
        "TRN2", target_bir_lowering=False, debug=False, num_devices=N_CORES)
    pf = nc.dram_tensor("pf", [C, NSH], f32, kind="ExternalInput")
    wvx = nc.dram_tensor("wvx", [C, 2 * AGE + 1], f32, kind="ExternalInput")
    iden = nc.dram_tensor("iden", [128, 128], f32, kind="ExternalInput")
    out = nc.dram_tensor("out", [NSH, C], f32, kind="ExternalOutput")

    with ExitStack() as ctx:
        e = ctx.enter_context
        sid = e(nc.semaphore("sid"))
        swx = e(nc.semaphore("swx"))
        spfA = e(nc.semaphore("spfA"))
        spfB = e(nc.semaphore("spfB"))
        sv1 = e(nc.semaphore("sv1"))
        sv2 = e(nc.semaphore("sv2"))
        svcol = e(nc.semaphore("svcol"))
        sadd = [e(nc.semaphore(f"sadd{k}")) for k in range(NCK)]
        spe = [e(nc.semaphore(f"spe{k}")) for k in range(NCK)]
        scp = [e(nc.semaphore(f"scp{k}")) for k in range(NCK)]
        sout = e(nc.semaphore("sout"))

        identsb = e(nc.sbuf_tensor("identsb", [128, 128], f32))
        wvxsb = e(nc.sbuf_tensor("wvxsb", [C, 2 * AGE + 1], f32))
        tmp = e(nc.sbuf_tensor("tmp", [C, AGE], f32))
        vsum = e(nc.sbuf_tensor("vsum", [C, 1], f32))
        vcol = e(nc.sbuf_tensor("vcol", [C, 1], f32))
        pft = e(nc.sbuf_tensor("pft", [C, NSH], f32))
        osb = [e(nc.sbuf_tensor(f"osb{k}", [128, CHUNKS[k]], f32))
               for k in range(NCK)]
        pgs = [e(nc.psum_tensor(f"pg{k}", [128, CHUNKS[k]], f32))
               for k in range(NCK)]
        block = e(nc.Block())

        def out_view(k):
            c0, w = COFF[k], CHUNKS[k]
            return out[c0:c0 + w, :].rearrange("(j p) c -> p j c", p=128)

        def osb_view(k):
            return osb[k][:].rearrange("p (j c) -> p j c", c=128)

        @block.sync
        def _(sync):
            sync.dma_start(
                out=pft[:, 0:HLF], in_=pf[:, 0:HLF]).then_inc(spfA, 16)
            sync.dma_start(
                out=pft[:, HLF:NSH], in_=pf[:, HLF:NSH]).then_inc(spfB, 16)
            for k in (0, 2):
                sync.wait_ge(scp[k], 1)
                sync.dma_start(
                    out=out_view(k), in_=osb_view(k)).then_inc(sout, 16)
            sync.wait_ge(sout, 16 * NCK)

        @block.scalar
        def _(scalar):
            scalar.dma_start(out=wvxsb[:], in_=wvx[:]).then_inc(swx, 16)
            scalar.dma_start(out=identsb[:], in_=iden[:]).then_inc(sid, 16)
            for k in (1, 3, 4):
                scalar.wait_ge(scp[k], 1)
                scalar.dma_start(
                    out=out_view(k), in_=osb_view(k)).then_inc(sout, 16)

        @block.tensor
        def _(tensor):
            tensor.wait_ge(sid, 16)
            for k in range(NCK):
                tensor.wait_ge(sadd[k], 1)
                for j in range(CHUNKS[k] // 128):
                    c0 = COFF[k] + j * 128
                    ins = tensor.transpose(
                        pgs[k][:, j * 128:(j + 1) * 128],
                        pft[:, c0:c0 + 128],
                        identsb[:],
                    )
                    if j == CHUNKS[k] // 128 - 1:
                        ins.then_inc(spe[k], 1)

        @block.vector
        def _(vector):
            import concourse.mybir as mybir

            vector.wait_ge(swx, 16)
            vector.tensor_tensor(
                tmp[:], wvxsb[:, 0:AGE], wvxsb[:, AGE:2 * AGE],
                mybir.AluOpType.mult).then_inc(sv1, 1)
            vector.wait_ge(sv1, 1)
            vector.reduce_sum(
                vsum[:], tmp[:], axis=mybir.AxisListType.X).then_inc(sv2, 1)
            vector.wait_ge(sv2, 1)
            vector.tensor_scalar(
                out=vcol[:], in0=vsum[:],
                scalar1=wvxsb[:, 2 * AGE:2 * AGE + 1], scalar2=None,
                op0=mybir.AluOpType.add,
            ).then_inc(svcol, 1)
            vector.wait_ge(svcol, 1)

            def pre_add(k):
                c0, w = COFF[k], CHUNKS[k]
                vector.wait_ge(spfA if k < 2 else spfB, 16)
                vector.tensor_scalar(
                    out=pft[:, c0:c0 + w], in0=pft[:, c0:c0 + w],
                    scalar1=vcol[:], scalar2=None,
                    op0=mybir.AluOpType.add,
                ).then_inc(sadd[k], 1)

            def copy_out(k):
                vector.wait_ge(spe[k], 1)
                vector.tensor_copy(osb[k][:], pgs[k][:]).then_inc(scp[k], 1)

            pre_add(0)
            pre_add(1)
            copy_out(0)
            pre_add(2)
            pre_add(3)
            pre_add(4)
            copy_out(1)
            copy_out(2)
            copy_out(3)
            copy_out(4)

    nc.finalize()
    return nc


_CACHE = {}
LAST_RESULTS = None


def kernel(**inputs):
    global LAST_RESULTS
    from concourse.bass_utils import run_bass_kernel_spmd

    if "nc" not in _CACHE:
        _CACHE["nc"] = build_nc()
    nc = _CACHE["nc"]

    pf_full = np.ascontiguousarray(
        np.asarray(inputs["pixel_features"], dtype=np.float32).reshape(C, N))
    age = np.asarray(inputs["age_features"], dtype=np.float32).reshape(AGE)
    wvx_np = np.empty((C, 2 * AGE + 1), dtype=np.float32)
    wvx_np[:, 0:AGE] = np.asarray(inputs["Wv"], dtype=np.float32)
    wvx_np[:, AGE:2 * AGE] = age[None, :]
    wvx_np[:, 2 * AGE] = np.asarray(inputs["bv"], dtype=np.float32)
    iden_np = np.eye(128, dtype=np.float32)

    in_maps = [
        {
            "pf": np.ascontiguousarray(pf_full[:, i * NSH:(i + 1) * NSH]),
            "wvx": wvx_np,
            "iden": iden_np,
        }
        for i in range(N_CORES)
    ]
    res = run_bass_kernel_spmd(nc, in_maps, core_ids=list(range(N_CORES)))
    LAST_RESULTS = res
    out = np.concatenate([res.results[i]["out"] for i in range(N_CORES)], axis=0)
    return out.reshape(B, N, C).astype(np.float32)


# revision 15
# speedup vs baseline: 1.1337x; 1.1337x over previous
"""Raw-Bacc v8: two-ring quarter loads (bulk first, gating loads early
on the other ring), DVE pre-add (+v), PE warmup to ramp p-state,
shrinking chunk tail (512,512,512,384,128) with per-chunk out DMAs.

out[n, c] = pf[c, n] + v[c],  v = Wv @ age + bv

wvx host-packed [128, 129] f32: cols 0:64 = Wv, 64:128 = age bcast,
col 128 = bv.
DVE: vcol = reduce_sum(Wv*age_bc) + bv; per chunk pre-add vcol into
pft (v is per-partition in [c, n] layout), PE transposes 128-col
blocks into the chunk's psum bank, DVE copies bank -> osb, out DMA
issued immediately, alternating rings.
Ring q1 (sync): pf0 (cols 0:512), pf2 (1024:1536), outs 0, 2, 4.
Ring q10 (scalar): wvx, iden, pf1 (512:1024), pf3 (1536:2048),
outs 1, 3.
PE: warmup transposes on wvxsb data after swx to ramp the clock while
loads stream.
"""

import numpy as np

N_CORES = 8
B, C, D, H, W = 1, 128, 16, 32, 32
N = D * H * W
NSH = N // N_CORES       # 2048
AGE = 64
QTR = 512
CHUNKS = [512, 512, 512, 384, 128]
NCK = len(CHUNKS)
COFF = [0, 512, 1024, 1536, 1920]
LOADQ = [0, 1, 2, 3, 3]   # which quarter-load sem gates each chunk
WARMUP = 9


def build_nc():
    import concourse.bacc as bacc
    import concourse.mybir as mybir
    from contextlib import ExitStack

    f32 = mybir.dt.float32
    nc = bacc.Bacc(
        "TRN2", target_bir_lowering=False, debug=False, num_devices=N_CORES)
    pf = nc.dram_tensor("pf", [C, NSH], f32, kind="ExternalInput")
    wvx = nc.dram_tensor("wvx", [C, 2 * AGE + 1], f32, kind="ExternalInput")
    iden = nc.dram_tensor("iden", [128, 128], f32, kind="ExternalInput")
    out = nc.dram_tensor("out", [NSH, C], f32, kind="ExternalOutput")

    with ExitStack() as ctx:
        e = ctx.enter_context
        sid = e(nc.semaphore("sid"))
        swx = e(nc.semaphore("swx"))
        spf = [e(nc.semaphore(f"spf{q}")) for q in range(4)]
        sv1 = e(nc.semaphore("sv1"))
        sv2 = e(nc.semaphore("sv2"))
        svcol = e(nc.semaphore("svcol"))
        sadd = [e(nc.semaphore(f"sadd{k}")) for k in range(NCK)]
        spe = [e(nc.semaphore(f"spe{k}")) for k in range(NCK)]
        scp = [e(nc.semaphore(f"scp{k}")) for k in range(NCK)]
        sout = e(nc.semaphore("sout"))

        identsb = e(nc.sbuf_tensor("identsb", [128, 128], f32))
        wvxsb = e(nc.sbuf_tensor("wvxsb", [C, 2 * AGE + 1], f32))
        tmp = e(nc.sbuf_tensor("tmp", [C, AGE], f32))
        vsum = e(nc.sbuf_tensor("vsum", [C, 1], f32))
        vcol = e(nc.sbuf_tensor("vcol", [C, 1], f32))
        pft = e(nc.sbuf_tensor("pft", [C, NSH], f32))
        osb = [e(nc.sbuf_tensor(f"osb{k}", [128, CHUNKS[k]], f32))
               for k in range(NCK)]
        pgw = e(nc.psum_tensor("pgw", [128, QTR], f32))
        pgs = [e(nc.psum_tensor(f"pg{k}", [128, CHUNKS[k]], f32))
               for k in range(NCK)]
        block = e(nc.Block())

        def out_view(k):
            c0, w = COFF[k], CHUNKS[k]
            return out[c0:c0 + w, :].rearrange("(j p) c -> p j c", p=128)

        def osb_view(k):
            return osb[k][:].rearrange("p (j c) -> p j c", c=128)

        @block.sync
        def _(sync):
            sync.dma_start(
                out=pft[:, 0:QTR], in_=pf[:, 0:QTR]).then_inc(spf[0], 16)
            sync.dma_start(
                out=pft[:, 2 * QTR:3 * QTR],
                in_=pf[:, 2 * QTR:3 * QTR]).then_inc(spf[2], 16)
            for k in (0, 2, 4):
                sync.wait_ge(scp[k], 1)
                sync.dma_start(
                    out=out_view(k), in_=osb_view(k)).then_inc(sout, 16)
            sync.wait_ge(sout, 16 * NCK)

        @block.scalar
        def _(scalar):
            scalar.dma_start(out=wvxsb[:], in_=wvx[:]).then_inc(swx, 16)
            scalar.dma_start(out=identsb[:], in_=iden[:]).then_inc(sid, 16)
            scalar.dma_start(
                out=pft[:, QTR:2 * QTR],
                in_=pf[:, QTR:2 * QTR]).then_inc(spf[1], 16)
            scalar.dma_start(
                out=pft[:, 3 * QTR:4 * QTR],
                in_=pf[:, 3 * QTR:4 * QTR]).then_inc(spf[3], 16)
            for k in (1, 3):
                scalar.wait_ge(scp[k], 1)
                scalar.dma_start(
                    out=out_view(k), in_=osb_view(k)).then_inc(sout, 16)

        @block.tensor
        def _(tensor):
            tensor.wait_ge(swx, 16)
            for i in range(WARMUP):
                tensor.transpose(
                    pgw[:, (i % 4) * 128:(i % 4 + 1) * 128],
                    wvxsb[:, 0:128], wvxsb[:, 0:128])
            tensor.wait_ge(sid, 16)
            for k in range(NCK):
                tensor.wait_ge(sadd[k], 1)
                for j in range(CHUNKS[k] // 128):
                    c0 = COFF[k] + j * 128
                    ins = tensor.transpose(
                        pgs[k][:, j * 128:(j + 1) * 128],
                        pft[:, c0:c0 + 128],
                        identsb[:],
                    )
                    if j == CHUNKS[k] // 128 - 1:
                        ins.then_inc(spe[k], 1)

        @block.vector
        def _(vector):
            import concourse.mybir as mybir

            vector.wait_ge(swx, 16)
            vector.tensor_tensor(
                tmp[:], wvxsb[:, 0:AGE], wvxsb[:, AGE:2 * AGE],
                mybir.AluOpType.mult).then_inc(sv1, 1)
            vector.wait_ge(sv1, 1)
            vector.reduce_sum(
                vsum[:], tmp[:], axis=mybir.AxisListType.X).then_inc(sv2, 1)
            vector.wait_ge(sv2, 1)
            vector.tensor_scalar(
                out=vcol[:], in0=vsum[:],
                scalar1=wvxsb[:, 2 * AGE:2 * AGE + 1], scalar2=None,
                op0=mybir.AluOpType.add,
            ).then_inc(svcol, 1)
            vector.wait_ge(svcol, 1)

            def pre_add(k):
                c0, w = COFF[k], CHUNKS[k]
                vector.wait_ge(spf[LOADQ[k]], 16)
                vector.tensor_scalar(
                    out=pft[:, c0:c0 + w], in0=pft[:, c0:c0 + w],
                    scalar1=vcol[:], scalar2=None,
                    op0=mybir.AluOpType.add,
                ).then_inc(sadd[k], 1)

            def copy_out(k):
                vector.wait_ge(spe[k], 1)
                vector.tensor_copy(osb[k][:], pgs[k][:]).then_inc(scp[k], 1)

            pre_add(0)
            pre_add(1)
            copy_out(0)
            pre_add(2)
            copy_out(1)
            pre_add(3)
            pre_add(4)
            copy_out(2)
            copy_out(3)
            copy_out(4)

    nc.finalize()
    return nc


_CACHE = {}
LAST_RESULTS = None


def kernel(**inputs):
    global LAST_RESULTS
    from concourse.bass_utils import run_bass_kernel_spmd

    if "nc" not in _CACHE:
        _CACHE["nc"] = build_nc()
    nc = _CACHE["nc"]

    pf_full = np.ascontiguousarray(
        np.asarray(inputs["pixel_features"], dtype=np.float32).reshape(C, N))
    age = np.asarray(inputs["age_features"], dtype=np.float32).reshape(AGE)
    wvx_np = np.empty((C, 2 * AGE + 1), dtype=np.float32)
    wvx_np[:, 0:AGE] = np.asarray(inputs["Wv"], dtype=np.float32)
    wvx_np[:, AGE:2 * AGE] = age[None, :]
    wvx_np[:, 2 * AGE] = np.asarray(inputs["bv"], dtype=np.float32)
    iden_np = np.eye(128, dtype=np.float32)

    in_maps = [
        {
            "pf": np.ascontiguousarray(pf_full[:, i * NSH:(i + 1) * NSH]),
            "wvx": wvx_np,
            "iden": iden_np,
        }
        for i in range(N_CORES)
    ]
    res = run_bass_kernel_spmd(nc, in_maps, core_ids=list(range(N_CORES)))
    LAST_RESULTS = res
    out = np.concatenate([res.results[i]["out"] for i in range(N_CORES)], axis=0)
    return out.reshape(B, N, C).astype(np.float32)
